# revision 12
# baseline (speedup 1.0000x reference)
"""Trainium2 Bass kernel for nn_Connection_v5extend (8-core data-parallel).

kernel(**inputs) takes the FULL unsharded inputs (as produced by
setup_inputs) and returns the FULL [4096, 256] float32 output.

Strategy: shard the batch dim (4096) into 8 contiguous row blocks, one per
NeuronCore; replicate the tiny MLP weights.  The reference's [B, D, D]
Jacobian is never materialized — the output reduces to 6 small matmuls plus
elementwise work:

    x, v  = input[:, :D], input[:, D:]
    h     = x @ W1.T + b1;  mask = (h > 0);  a = relu(h)
    s     = sigmoid(a @ W2.T + b2)
    nsig  = (s - 1) * s              # = -s(1-s)
    p     = (v^2 * (-sgn)) * nsig    # = v^2 * sgn * sig
    term1 = ((p @ W2) * mask) @ W1
    T2p   = (mask * (v @ W1.T)) @ W2.T
    dv    = (-1/(0.5 s + C/2)) * ((v * nsig) * T2p + 0.5*sgn*term1)
    out   = concat([v, dv], axis=1)

On-chip layout is feature-major (features on SBUF partitions, batch on the
free dim); x/v are PE-transposed on load and dv transposed back on store.
All matmuls run in float32r via bitcast views (fp32 storage, 1 col/cycle).
The PE instruction stream interleaves the two batch segments so the tensor
engine never idles long enough for the HAM clock gate to re-throttle.
"""

import sys

sys.path.insert(0, "/opt/trn_rl_repo")

import numpy as np

import concourse.bass as bass  # noqa: F401
import concourse.bacc as bacc
import concourse.mybir as mybir
import concourse.tile as tile
from concourse.masks import make_identity
from concourse.bass_utils import run_bass_kernel_spmd

F32 = mybir.dt.float32
F32R = mybir.dt.float32r
AF = mybir.ActivationFunctionType
ALU = mybir.AluOpType

D = 128
CONST = 0.618
SIGN = 4
N_CORES = 8
BATCH = 4096
B = BATCH // N_CORES  # rows per core


def _build(nc, B=512, n_seg=2, h_dtype="float32r"):
    NCH = B // D           # 128-row chunks for PE transposes
    SEG = B // n_seg       # batch columns per pipeline segment
    CPS = SEG // D         # chunks per segment
    assert NCH * D == B and SEG * n_seg == B and CPS * D == SEG

    inp = nc.dram_tensor("inp", [B, 2 * D], F32, kind="ExternalInput").ap()
    W1 = nc.dram_tensor("W1", [2 * D, D], F32, kind="ExternalInput").ap()
    b1 = nc.dram_tensor("b1", [2 * D], F32, kind="ExternalInput").ap()
    W2 = nc.dram_tensor("W2", [D, 2 * D], F32, kind="ExternalInput").ap()
    b2 = nc.dram_tensor("b2", [D], F32, kind="ExternalInput").ap()
    out = nc.dram_tensor("out", [B, 2 * D], F32, kind="ExternalOutput").ap()

    def r(ap):
        return ap.bitcast(F32R)

    with tile.TileContext(nc) as tc:
        with (
            tc.tile_pool(name="consts", bufs=1) as consts,
            tc.tile_pool(name="sb", bufs=1) as sb,
            tc.tile_pool(name="seg", bufs=2) as sgp,
            tc.tile_pool(name="ps", bufs=2, space="PSUM") as ps,
            tc.tile_pool(name="pss", bufs=2, space="PSUM") as pss,
            tc.tile_pool(name="pst", bufs=2, space="PSUM") as pst,
        ):
            # ---------------- DMAs first on every queue ----------------
            # DMA issue costs ~0.65us of engine time each, so: the scalar
            # engine (which owns all activations) gets only the two input
            # loads; everything else rides the sync ring or SWDGE.
            it = sb.tile([D, NCH, 2 * D], F32, tag="it", name="it")
            nc.scalar.dma_start(
                it[:, 0:2, :],
                inp.rearrange("(c p) f -> p c f", p=D)[:, 0:2, :])
            nc.scalar.dma_start(
                it[:, 2:4, :],
                inp.rearrange("(c p) f -> p c f", p=D)[:, 2:4, :])

            w1k = consts.tile([D, 2, D], F32, tag="w1k", name="w1k")
            nc.sync.dma_start(w1k[:], W1.rearrange("(h p) j -> p h j", p=D))
            w2n = consts.tile([D, 2 * D], F32, tag="w2n", name="w2n")
            nc.sync.dma_start(w2n[:], W2)
            b1c = consts.tile([D, 2], F32, tag="b1c", name="b1c")
            nc.sync.dma_start(b1c[:], b1.rearrange("(h p) -> p h", p=D))
            b2c = consts.tile([D, 1], F32, tag="b2c", name="b2c")
            nc.sync.dma_start(b2c[:], b2.rearrange("(p o) -> p o", o=1))
            # v passthrough: HBM->HBM, independent of all compute
            nc.sync.dma_start(out[:, 0:D], inp[:, D:2 * D])

            # ---------------- constants ----------------
            ident = consts.tile([D, D], F32, tag="ident", name="ident")
            make_identity(nc, ident[:])

            # sgnc = sgn; nhsgn = -0.5*sgn; c309 = CONST/2
            sgnc = consts.tile([D, 1], F32, tag="sgnc", name="sgnc")
            nc.vector.memset(sgnc[:], 1.0)
            nc.vector.memset(sgnc[:SIGN, :], -1.0)
            nhsgn = consts.tile([D, 1], F32, tag="nhsgn", name="nhsgn")
            nc.vector.memset(nhsgn[:], -0.5)
            nc.vector.memset(nhsgn[:SIGN, :], 0.5)
            c309 = consts.tile([D, 1], F32, tag="c309", name="c309")
            nc.vector.memset(c309[:], CONST / 2.0)

            # f32r copies of the DMA-loaded weights (t1/u matmul lhsT);
            # w2s folds the per-row sgn into W2 so p needs no sign factor
            w1kr = consts.tile([D, 2, D], F32R, tag="w1kr", name="w1kr")
            w2s = consts.tile([D, 2 * D], F32R, tag="w2s", name="w2s")

            def g_wcast():
                nc.gpsimd.tensor_copy(w1kr[:], w1k[:])

            def s_w2s():
                nc.scalar.mul(w2s[:], w2n[:], sgnc[:, 0:1])

            # ---------------- weight transposes (PE) ----------------
            w1T = consts.tile([D, 2, D], F32R, tag="w1T", name="w1T")
            w2T = consts.tile([D, 2, D], F32R, tag="w2T", name="w2T")

            def tw1():
                tp = pst.tile([D, 2, D], F32, tag="itp", name="itp")
                nc.tensor.transpose(tp[:, 0, :], w1k[:, 0, :], ident[:])
                nc.tensor.transpose(tp[:, 1, :], w1k[:, 1, :], ident[:])
                nc.vector.tensor_copy(w1T[:], tp[:])

            def tw2():
                tp = pst.tile([D, 2, D], F32, tag="itp", name="itp")
                nc.tensor.transpose(tp[:, 0, :], w2n[:, 0:D], ident[:])
                nc.tensor.transpose(tp[:, 1, :], w2n[:, D:2 * D], ident[:])
                nc.vector.tensor_copy(w2T[:], tp[:])

            # ---------------- input transposes (PE) ----------------
            xvT = sb.tile([D, 2, B], F32R, tag="xvT", name="xvT")

            def tin(c):
                itp = pst.tile([D, 2, D], F32, tag="itp", name="itp")
                nc.tensor.transpose(itp[:, 0, :], it[:, c, 0:D], ident[:])
                nc.tensor.transpose(itp[:, 1, :], it[:, c, D:2 * D], ident[:])
                nc.vector.tensor_copy(xvT[:, :, c * D:(c + 1) * D], itp[:])

            # ---------------- per-segment stages ----------------
            hps = [None] * n_seg
            wps = [None] * n_seg
            zps = [None] * n_seg
            ups = [None] * n_seg
            t1ps = [None] * n_seg
            t2ps = [None] * n_seg
            a_ = [None] * n_seg
            mask = [None] * n_seg
            s_ = [None] * n_seg
            sig = [None] * n_seg
            ssq = [None] * n_seg
            vsq = [None] * n_seg
            p_ = [None] * n_seg
            r_ = [None] * n_seg
            mw = [None] * n_seg
            vs = [None] * n_seg
            a2 = [None] * n_seg
            cc = [None] * n_seg
            rp = [None] * n_seg
            r02 = [None] * n_seg
            dv = [None] * n_seg

            def xT(g):
                return xvT[:, 0, g * SEG:(g + 1) * SEG]

            def vT(g):
                return xvT[:, 1, g * SEG:(g + 1) * SEG]

            def vTf(g):
                return vT(g).bitcast(F32)

            # PE stages
            def mm_h(g):
                hps[g] = ps.tile([D, 2, SEG], F32, tag="big", name="big")
                nc.tensor.matmul(hps[g][:, 0, :], w1T[:, 0, :], xT(g))
                nc.tensor.matmul(hps[g][:, 1, :], w1T[:, 1, :], xT(g))

            def mm_w(g):
                wps[g] = ps.tile([D, 2, SEG], F32, tag="wp", name="wp")
                nc.tensor.matmul(wps[g][:, 0, :], w1T[:, 0, :], vT(g))
                nc.tensor.matmul(wps[g][:, 1, :], w1T[:, 1, :], vT(g))

            def mm_z(g):
                zps[g] = pss.tile([D, SEG], F32, tag="small", name="small")
                nc.tensor.matmul(zps[g][:], w2T[:, 0, :], a_[g][:, 0, :],
                                 start=True, stop=False)
                nc.tensor.matmul(zps[g][:], w2T[:, 1, :], a_[g][:, 1, :],
                                 start=False, stop=True)

            def mm_u(g):
                ups[g] = ps.tile([D, 2, SEG], F32, tag="big", name="big")
                nc.tensor.matmul(ups[g][:, 0, :], w2s[:, 0:D], p_[g][:])
                nc.tensor.matmul(ups[g][:, 1, :], w2s[:, D:2 * D], p_[g][:])

            def mm_t1(g):
                t1ps[g] = pss.tile([D, SEG], F32, tag="small", name="small")
                nc.tensor.matmul(t1ps[g][:], w1kr[:, 0, :], r_[g][:, 0, :],
                                 start=True, stop=False)
                nc.tensor.matmul(t1ps[g][:], w1kr[:, 1, :], r_[g][:, 1, :],
                                 start=False, stop=True)

            def mm_t2(g):
                t2ps[g] = pss.tile([D, SEG], F32, tag="small", name="small")
                nc.tensor.matmul(t2ps[g][:], w2T[:, 0, :], mw[g][:, 0, :],
                                 start=True, stop=False)
                nc.tensor.matmul(t2ps[g][:], w2T[:, 1, :], mw[g][:, 1, :],
                                 start=False, stop=True)

            # scalar stages
            def s_relu(g):
                a_[g] = sgp.tile([D, 2, SEG], F32R, tag="a", name="a")
                for h in range(2):
                    nc.scalar.activation(a_[g][:, h, :], hps[g][:, h, :],
                                         AF.Relu, bias=b1c[:, h:h + 1])

            def s_sig(g):
                s_[g] = sgp.tile([D, SEG], F32, tag="s", name="s")
                nc.scalar.activation(s_[g][:], zps[g][:], AF.Sigmoid,
                                     bias=b2c[:, 0:1])

            def s_rp(g):
                # rp = 0.5 s + CONST/2; recip gives 1/g_abs
                rp[g] = sgp.tile([D, SEG], F32, tag="rp", name="rp")
                nc.scalar.activation(rp[g][:], s_[g][:], AF.Identity,
                                     bias=c309[:, 0:1], scale=0.5)

            def s_ssq(g):
                ssq[g] = sgp.tile([D, SEG], F32, tag="ssq", name="ssq")
                nc.scalar.square(ssq[g][:], s_[g][:])

            # gpsimd stages
            def g_vsq(g):
                vsq[g] = sgp.tile([D, SEG], F32, tag="vsq", name="vsq")
                nc.gpsimd.tensor_mul(vsq[g][:], vTf(g), vTf(g))

            def g_mask(g):
                # a = relu(h) >= 0, so (a > 0) == (h > 0)
                mask[g] = sgp.tile([D, 2, SEG], F32, tag="mask", name="mask")
                nc.gpsimd.tensor_single_scalar(
                    mask[g].rearrange("p s b -> p (s b)"),
                    a_[g].rearrange("p s b -> p (s b)").bitcast(F32), 0.0,
                    ALU.is_gt)

            def v_mw(g):
                mw[g] = sgp.tile([D, 2, SEG], F32R, tag="mw", name="mw")
                nc.vector.tensor_mul(mw[g].rearrange("p s b -> p (s b)"),
                                     wps[g].rearrange("p s b -> p (s b)"),
                                     mask[g].rearrange("p s b -> p (s b)"))

            def g_vs(g):
                vs[g] = sgp.tile([D, SEG], F32, tag="vs", name="vs")
                nc.gpsimd.tensor_mul(vs[g][:], vTf(g), sig[g][:])

            # vector stages
            def g_sig(g):
                sig[g] = sgp.tile([D, SEG], F32, tag="sig", name="sig")
                nc.gpsimd.tensor_sub(sig[g][:], s_[g][:], ssq[g][:])

            def g_p(g):
                # p = vsq * sig  (sgn folded into w2s)
                p_[g] = sgp.tile([D, SEG], F32R, tag="p", name="p")
                nc.gpsimd.tensor_mul(p_[g][:], vsq[g][:], sig[g][:])

            def g_dv(g):
                dv[g] = sgp.tile([D, SEG], F32, tag="dv", name="dv")
                nc.gpsimd.tensor_mul(dv[g][:], cc[g][:], r02[g][:])

            def v_r(g):
                r_[g] = sgp.tile([D, 2, SEG], F32R, tag="r", name="r")
                nc.vector.tensor_mul(r_[g].rearrange("p s b -> p (s b)"),
                                     ups[g].rearrange("p s b -> p (s b)"),
                                     mask[g].rearrange("p s b -> p (s b)"))

            def v_recip(g):
                r02[g] = sgp.tile([D, SEG], F32, tag="r02", name="r02")
                nc.vector.reciprocal_approx_fast(r02[g][:], rp[g][:])

            def v_a2(g):
                a2[g] = sgp.tile([D, SEG], F32, tag="a2", name="a2")
                nc.vector.tensor_mul(a2[g][:], vs[g][:], t2ps[g][:])

            def v_cc(g):
                # cc = (t1 * (-0.5 sgn)) + v sig T2p
                cc[g] = sgp.tile([D, SEG], F32, tag="cc", name="cc")
                nc.vector.scalar_tensor_tensor(cc[g][:], t1ps[g][:],
                                               nhsgn[:, 0:1], a2[g][:],
                                               ALU.mult, ALU.add)

            # output transposes + stores
            ot = sb.tile([D, NCH, D], F32, tag="ot", name="ot")

            def tout(g):
                otp = pst.tile([D, 2, D], F32, tag="itp", name="itp")
                for k in range(CPS):
                    nc.tensor.transpose(otp[:, k, :],
                                        dv[g][:, k * D:(k + 1) * D], ident[:])
                nc.scalar.copy(ot[:, g * CPS:(g + 1) * CPS, :], otp[:])
                eng = nc.sync if g % 2 == 0 else nc.scalar
                eng.dma_start(
                    out.rearrange("(c p) f -> p c f", p=D)
                    [:, g * CPS:(g + 1) * CPS, D:2 * D],
                    ot[:, g * CPS:(g + 1) * CPS, :])

            # ---------------- schedule (program order per engine) -------
            tw1()
            g_wcast()
            tin(0)
            tin(1)
            s_w2s()
            mm_h(0)
            tw2()
            tin(2)
            tin(3)
            s_relu(0)
            g_vsq(0)
            mm_h(1)
            g_mask(0)
            mm_z(0)
            s_relu(1)
            mm_w(0)
            g_vsq(1)
            s_sig(0)
            s_ssq(0)
            g_mask(1)
            g_sig(0)
            g_p(0)
            mm_u(0)
            v_mw(0)
            mm_z(1)
            s_sig(1)
            s_ssq(1)
            mm_w(1)
            s_rp(0)
            g_sig(1)
            g_vs(0)
            g_p(1)
            v_recip(0)
            v_r(0)
            mm_u(1)
            mm_t1(0)
            mm_t2(0)
            s_rp(1)
            g_vs(1)
            v_mw(1)
            v_r(1)
            mm_t1(1)
            mm_t2(1)
            v_a2(0)
            v_cc(0)
            g_dv(0)
            tout(0)
            v_recip(1)
            v_a2(1)
            v_cc(1)
            g_dv(1)
            tout(1)

    return nc


_CACHE = {}


def _get_nc(variant="v2"):
    if variant not in _CACHE:
        nc = bacc.Bacc("TRN2", target_bir_lowering=False, debug=False,
                       num_devices=N_CORES)
        _build(nc, B=B)
        nc.compile()
        _CACHE[variant] = nc
    return _CACHE[variant]


def kernel(t, input_, W1, b1, W2, b2):
    input_ = np.ascontiguousarray(np.asarray(input_, dtype=np.float32))
    W1 = np.ascontiguousarray(np.asarray(W1, dtype=np.float32))
    b1 = np.ascontiguousarray(np.asarray(b1, dtype=np.float32))
    W2 = np.ascontiguousarray(np.asarray(W2, dtype=np.float32))
    b2 = np.ascontiguousarray(np.asarray(b2, dtype=np.float32))
    assert input_.shape == (BATCH, 2 * D)

    nc = _get_nc()
    in_maps = [
        {"inp": input_[c * B:(c + 1) * B], "W1": W1, "b1": b1, "W2": W2, "b2": b2}
        for c in range(N_CORES)
    ]
    res = run_bass_kernel_spmd(nc, in_maps, core_ids=list(range(N_CORES)))
    return np.concatenate([res.results[c]["out"] for c in range(N_CORES)], axis=0)


# revision 15
# speedup vs baseline: 1.3683x; 1.3683x over previous
"""Trainium2 Bass kernel for nn_Connection_v5extend (8-core data-parallel).

kernel(**inputs) takes the FULL unsharded inputs (as produced by
setup_inputs) and returns the FULL [4096, 256] float32 output.

Strategy: shard the batch dim (4096) into 8 contiguous row blocks, one per
NeuronCore; replicate the tiny MLP weights.  The reference's [B, D, D]
Jacobian is never materialized — the output reduces to 6 small matmuls plus
elementwise work:

    x, v  = input[:, :D], input[:, D:]
    h     = x @ W1.T + b1;  mask = (h > 0);  a = relu(h)
    s     = sigmoid(a @ W2.T + b2)
    nsig  = (s - 1) * s              # = -s(1-s)
    p     = (v^2 * (-sgn)) * nsig    # = v^2 * sgn * sig
    term1 = ((p @ W2) * mask) @ W1
    T2p   = (mask * (v @ W1.T)) @ W2.T
    dv    = (-1/(0.5 s + C/2)) * ((v * nsig) * T2p + 0.5*sgn*term1)
    out   = concat([v, dv], axis=1)

On-chip layout is feature-major (features on SBUF partitions, batch on the
free dim); x/v are PE-transposed on load and dv transposed back on store.
All matmuls run in float32r via bitcast views (fp32 storage, 1 col/cycle).
The PE instruction stream interleaves the two batch segments so the tensor
engine never idles long enough for the HAM clock gate to re-throttle.
"""

import sys

sys.path.insert(0, "/opt/trn_rl_repo")

import numpy as np

import concourse.bass as bass  # noqa: F401
import concourse.bacc as bacc
import concourse.mybir as mybir
import concourse.tile as tile
from concourse.masks import make_identity
from concourse.bass_utils import run_bass_kernel_spmd

F32 = mybir.dt.float32
F32R = mybir.dt.float32r
AF = mybir.ActivationFunctionType
ALU = mybir.AluOpType

D = 128
CONST = 0.618
SIGN = 4
N_CORES = 8
BATCH = 4096
B = BATCH // N_CORES  # rows per core


def _build(nc, B=512, n_seg=2, h_dtype="float32r"):
    NCH = B // D           # 128-row chunks for PE transposes
    SEG = B // n_seg       # batch columns per pipeline segment
    CPS = SEG // D         # chunks per segment
    assert NCH * D == B and SEG * n_seg == B and CPS * D == SEG

    inp = nc.dram_tensor("inp", [B, 2 * D], F32, kind="ExternalInput").ap()
    W1 = nc.dram_tensor("W1", [2 * D, D], F32, kind="ExternalInput").ap()
    b1 = nc.dram_tensor("b1", [2 * D], F32, kind="ExternalInput").ap()
    W2 = nc.dram_tensor("W2", [D, 2 * D], F32, kind="ExternalInput").ap()
    b2 = nc.dram_tensor("b2", [D], F32, kind="ExternalInput").ap()
    out = nc.dram_tensor("out", [B, 2 * D], F32, kind="ExternalOutput").ap()

    def r(ap):
        return ap.bitcast(F32R)

    with tile.TileContext(nc) as tc:
        with (
            tc.tile_pool(name="consts", bufs=1) as consts,
            tc.tile_pool(name="sb", bufs=1) as sb,
            tc.tile_pool(name="seg", bufs=2) as sgp,
            tc.tile_pool(name="ps", bufs=2, space="PSUM") as ps,
            tc.tile_pool(name="pss", bufs=2, space="PSUM") as pss,
            tc.tile_pool(name="pst", bufs=2, space="PSUM") as pst,
        ):
            # ---------------- DMAs first on every queue ----------------
            # DMA issue costs ~0.65us of engine time each, so: the scalar
            # engine (which owns all activations) gets only the two input
            # loads; everything else rides the sync ring or SWDGE.
            it = sb.tile([D, NCH, 2 * D], F32, tag="it", name="it")
            nc.scalar.dma_start(
                it[:, 0:2, :],
                inp.rearrange("(c p) f -> p c f", p=D)[:, 0:2, :])
            nc.scalar.dma_start(
                it[:, 2:4, :],
                inp.rearrange("(c p) f -> p c f", p=D)[:, 2:4, :])

            w1k = consts.tile([D, 2, D], F32, tag="w1k", name="w1k")
            nc.sync.dma_start(w1k[:], W1.rearrange("(h p) j -> p h j", p=D))
            w2n = consts.tile([D, 2 * D], F32, tag="w2n", name="w2n")
            nc.sync.dma_start(w2n[:], W2)
            b1c = consts.tile([D, 2], F32, tag="b1c", name="b1c")
            nc.sync.dma_start(b1c[:], b1.rearrange("(h p) -> p h", p=D))
            b2c = consts.tile([D, 1], F32, tag="b2c", name="b2c")
            nc.sync.dma_start(b2c[:], b2.rearrange("(p o) -> p o", o=1))
            # v passthrough: HBM->HBM, independent of all compute
            nc.sync.dma_start(out[:, 0:D], inp[:, D:2 * D])

            # ---------------- constants ----------------
            ident = consts.tile([D, D], F32, tag="ident", name="ident")
            make_identity(nc, ident[:])

            # sgnc = sgn; nhsgn = -0.5*sgn; c309 = CONST/2
            sgnc = consts.tile([D, 1], F32, tag="sgnc", name="sgnc")
            nc.vector.memset(sgnc[:], 1.0)
            nc.vector.memset(sgnc[:SIGN, :], -1.0)
            nhsgn = consts.tile([D, 1], F32, tag="nhsgn", name="nhsgn")
            nc.vector.memset(nhsgn[:], -0.5)
            nc.vector.memset(nhsgn[:SIGN, :], 0.5)
            c309 = consts.tile([D, 1], F32, tag="c309", name="c309")
            nc.vector.memset(c309[:], CONST / 2.0)

            # f32r copies of the DMA-loaded weights (t1/u matmul lhsT);
            # w2s folds the per-row sgn into W2 so p needs no sign factor
            w1kr = consts.tile([D, 2, D], F32R, tag="w1kr", name="w1kr")
            w2s = consts.tile([D, 2 * D], F32R, tag="w2s", name="w2s")

            def g_wcast():
                nc.gpsimd.tensor_copy(w1kr[:], w1k[:])

            def s_w2s():
                nc.scalar.mul(w2s[:], w2n[:], sgnc[:, 0:1])

            # ---------------- weight transposes (PE) ----------------
            w1T = consts.tile([D, 2, D], F32R, tag="w1T", name="w1T")
            w2T = consts.tile([D, 2, D], F32R, tag="w2T", name="w2T")

            def tw1():
                tp = pst.tile([D, 2, D], F32, tag="itp", name="itp")
                nc.tensor.transpose(tp[:, 0, :], w1k[:, 0, :], ident[:])
                nc.tensor.transpose(tp[:, 1, :], w1k[:, 1, :], ident[:])
                nc.vector.tensor_copy(w1T[:], tp[:])

            def tw2():
                tp = pst.tile([D, 2, D], F32, tag="itp", name="itp")
                nc.tensor.transpose(tp[:, 0, :], w2n[:, 0:D], ident[:])
                nc.tensor.transpose(tp[:, 1, :], w2n[:, D:2 * D], ident[:])
                nc.vector.tensor_copy(w2T[:], tp[:])

            # ---------------- input transposes (PE) ----------------
            xvT = sb.tile([D, 2, B], F32R, tag="xvT", name="xvT")

            def tin(c):
                itp = pst.tile([D, 2, D], F32, tag="itp", name="itp")
                nc.tensor.transpose(itp[:, 0, :], it[:, c, 0:D], ident[:])
                nc.tensor.transpose(itp[:, 1, :], it[:, c, D:2 * D], ident[:])
                if c < 2:
                    nc.scalar.copy(xvT[:, :, c * D:(c + 1) * D], itp[:])
                else:
                    nc.vector.tensor_copy(xvT[:, :, c * D:(c + 1) * D], itp[:])

            # ---------------- per-segment stages ----------------
            hps = [None] * n_seg
            wps = [None] * n_seg
            zps = [None] * n_seg
            ups = [None] * n_seg
            t1ps = [None] * n_seg
            t2ps = [None] * n_seg
            a_ = [None] * n_seg
            mask = [None] * n_seg
            s_ = [None] * n_seg
            sig = [None] * n_seg
            ssq = [None] * n_seg
            vsq = [None] * n_seg
            p_ = [None] * n_seg
            r_ = [None] * n_seg
            mw = [None] * n_seg
            vs = [None] * n_seg
            a2 = [None] * n_seg
            cc = [None] * n_seg
            rp = [None] * n_seg
            r02 = [None] * n_seg
            dv = [None] * n_seg

            def xT(g):
                return xvT[:, 0, g * SEG:(g + 1) * SEG]

            def vT(g):
                return xvT[:, 1, g * SEG:(g + 1) * SEG]

            def vTf(g):
                return vT(g).bitcast(F32)

            # PE stages
            def mm_h(g):
                hps[g] = ps.tile([D, 2, SEG], F32, tag="big", name="big")
                nc.tensor.matmul(hps[g][:, 0, :], w1T[:, 0, :], xT(g))
                nc.tensor.matmul(hps[g][:, 1, :], w1T[:, 1, :], xT(g))

            def mm_w(g):
                wps[g] = ps.tile([D, 2, SEG], F32, tag="wp", name="wp")
                nc.tensor.matmul(wps[g][:, 0, :], w1T[:, 0, :], vT(g))
                nc.tensor.matmul(wps[g][:, 1, :], w1T[:, 1, :], vT(g))

            def mm_z(g):
                zps[g] = pss.tile([D, SEG], F32, tag="small", name="small")
                nc.tensor.matmul(zps[g][:], w2T[:, 0, :], a_[g][:, 0, :],
                                 start=True, stop=False)
                nc.tensor.matmul(zps[g][:], w2T[:, 1, :], a_[g][:, 1, :],
                                 start=False, stop=True)

            def mm_u(g):
                ups[g] = ps.tile([D, 2, SEG], F32, tag="big", name="big")
                nc.tensor.matmul(ups[g][:, 0, :], w2s[:, 0:D], p_[g][:])
                nc.tensor.matmul(ups[g][:, 1, :], w2s[:, D:2 * D], p_[g][:])

            def mm_t1(g):
                t1ps[g] = pss.tile([D, SEG], F32, tag="small", name="small")
                nc.tensor.matmul(t1ps[g][:], w1kr[:, 0, :], r_[g][:, 0, :],
                                 start=True, stop=False)
                nc.tensor.matmul(t1ps[g][:], w1kr[:, 1, :], r_[g][:, 1, :],
                                 start=False, stop=True)

            def mm_t2(g):
                t2ps[g] = pss.tile([D, SEG], F32, tag="small", name="small")
                nc.tensor.matmul(t2ps[g][:], w2T[:, 0, :], mw[g][:, 0, :],
                                 start=True, stop=False)
                nc.tensor.matmul(t2ps[g][:], w2T[:, 1, :], mw[g][:, 1, :],
                                 start=False, stop=True)

            # scalar stages
            def s_relu(g):
                a_[g] = sgp.tile([D, 2, SEG], F32R, tag="a", name="a")
                for h in range(2):
                    nc.scalar.activation(a_[g][:, h, :], hps[g][:, h, :],
                                         AF.Relu, bias=b1c[:, h:h + 1])

            def s_sig(g):
                s_[g] = sgp.tile([D, SEG], F32, tag="s", name="s")
                nc.scalar.activation(s_[g][:], zps[g][:], AF.Sigmoid,
                                     bias=b2c[:, 0:1])

            def s_rp(g):
                # rp = 0.5 s + CONST/2; recip gives 1/g_abs
                rp[g] = sgp.tile([D, SEG], F32, tag="rp", name="rp")
                nc.scalar.activation(rp[g][:], s_[g][:], AF.Identity,
                                     bias=c309[:, 0:1], scale=0.5)

            def s_ssq(g):
                ssq[g] = sgp.tile([D, SEG], F32, tag="ssq", name="ssq")
                nc.scalar.square(ssq[g][:], s_[g][:])

            # gpsimd stages
            def v_mask(g):
                # a = relu(h) >= 0, so (a > 0) == (h > 0)
                mask[g] = sgp.tile([D, 2, SEG], F32, tag="mask", name="mask")
                nc.vector.tensor_single_scalar(
                    mask[g].rearrange("p s b -> p (s b)"),
                    a_[g].rearrange("p s b -> p (s b)").bitcast(F32), 0.0,
                    ALU.is_gt)

            def v_mw(g):
                mw[g] = sgp.tile([D, 2, SEG], F32R, tag="mw", name="mw")
                nc.vector.tensor_mul(mw[g].rearrange("p s b -> p (s b)"),
                                     wps[g].rearrange("p s b -> p (s b)"),
                                     mask[g].rearrange("p s b -> p (s b)"))

            def g_vs(g):
                vs[g] = sgp.tile([D, SEG], F32, tag="vs", name="vs")
                nc.gpsimd.tensor_mul(vs[g][:], vTf(g), sig[g][:])

            # vector stages
            def v_sig(g):
                sig[g] = sgp.tile([D, SEG], F32, tag="sig", name="sig")
                nc.vector.tensor_sub(sig[g][:], s_[g][:], ssq[g][:])

            def g_p(g):
                # p = v^2 sig = (v sig) * v  (sgn folded into w2s)
                p_[g] = sgp.tile([D, SEG], F32R, tag="p", name="p")
                nc.gpsimd.tensor_mul(p_[g][:], vs[g][:], vTf(g))

            def v_recip(g):
                r02[g] = sgp.tile([D, SEG], F32, tag="r02", name="r02")
                nc.vector.reciprocal_approx_fast(r02[g][:], rp[g][:])

            def v_dv(g):
                dv[g] = sgp.tile([D, SEG], F32, tag="dv", name="dv")
                nc.vector.tensor_mul(dv[g][:], cc[g][:], r02[g][:])

            def v_r(g):
                r_[g] = sgp.tile([D, 2, SEG], F32R, tag="r", name="r")
                nc.vector.tensor_mul(r_[g].rearrange("p s b -> p (s b)"),
                                     ups[g].rearrange("p s b -> p (s b)"),
                                     mask[g].rearrange("p s b -> p (s b)"))

            def v_a2(g):
                a2[g] = sgp.tile([D, SEG], F32, tag="a2", name="a2")
                nc.vector.tensor_mul(a2[g][:], vs[g][:], t2ps[g][:])

            def v_cc(g):
                # cc = (t1 * (-0.5 sgn)) + v sig T2p
                cc[g] = sgp.tile([D, SEG], F32, tag="cc", name="cc")
                nc.vector.scalar_tensor_tensor(cc[g][:], t1ps[g][:],
                                               nhsgn[:, 0:1], a2[g][:],
                                               ALU.mult, ALU.add)

            # output transposes + stores
            ot = sb.tile([D, NCH, D], F32, tag="ot", name="ot")

            def tout(g):
                otp = pst.tile([D, 2, D], F32, tag="itp", name="itp")
                for k in range(CPS):
                    nc.tensor.transpose(otp[:, k, :],
                                        dv[g][:, k * D:(k + 1) * D], ident[:])
                nc.scalar.copy(ot[:, g * CPS:(g + 1) * CPS, :], otp[:])
                eng = nc.sync if g % 2 == 0 else nc.scalar
                eng.dma_start(
                    out.rearrange("(c p) f -> p c f", p=D)
                    [:, g * CPS:(g + 1) * CPS, D:2 * D],
                    ot[:, g * CPS:(g + 1) * CPS, :])

            # ---------------- schedule (program order per engine) -------
            tw1()
            g_wcast()
            tin(0)
            tin(1)
            s_w2s()
            mm_h(0)
            tw2()
            tin(2)
            tin(3)
            s_relu(0)
            mm_h(1)
            v_mask(0)
            mm_z(0)
            s_relu(1)
            mm_w(0)
            s_sig(0)
            s_ssq(0)
            v_sig(0)
            g_vs(0)
            g_p(0)
            v_mw(0)
            mm_u(0)
            mm_z(1)
            s_sig(1)
            s_ssq(1)
            v_mask(1)
            s_rp(0)
            v_recip(0)
            v_r(0)
            mm_w(1)
            v_sig(1)
            g_vs(1)
            g_p(1)
            mm_u(1)
            mm_t1(0)
            mm_t2(0)
            s_rp(1)
            v_recip(1)
            v_mw(1)
            v_r(1)
            mm_t1(1)
            mm_t2(1)
            v_a2(0)
            v_cc(0)
            v_dv(0)
            tout(0)
            v_a2(1)
            v_cc(1)
            v_dv(1)
            tout(1)

    return nc


_CACHE = {}


def _get_nc(variant="v2"):
    if variant not in _CACHE:
        nc = bacc.Bacc("TRN2", target_bir_lowering=False, debug=False,
                       num_devices=N_CORES)
        _build(nc, B=B)
        nc.compile()
        _CACHE[variant] = nc
    return _CACHE[variant]


def kernel(t, input_, W1, b1, W2, b2):
    input_ = np.ascontiguousarray(np.asarray(input_, dtype=np.float32))
    W1 = np.ascontiguousarray(np.asarray(W1, dtype=np.float32))
    b1 = np.ascontiguousarray(np.asarray(b1, dtype=np.float32))
    W2 = np.ascontiguousarray(np.asarray(W2, dtype=np.float32))
    b2 = np.ascontiguousarray(np.asarray(b2, dtype=np.float32))
    assert input_.shape == (BATCH, 2 * D)

    nc = _get_nc()
    in_maps = [
        {"inp": input_[c * B:(c + 1) * B], "W1": W1, "b1": b1, "W2": W2, "b2": b2}
        for c in range(N_CORES)
    ]
    res = run_bass_kernel_spmd(nc, in_maps, core_ids=list(range(N_CORES)))
    return np.concatenate([res.results[c]["out"] for c in range(N_CORES)], axis=0)


# revision 17
# speedup vs baseline: 1.4408x; 1.0530x over previous
"""Trainium2 Bass kernel for nn_Connection_v5extend (8-core data-parallel).

kernel(**inputs) takes the FULL unsharded inputs (as produced by
setup_inputs) and returns the FULL [4096, 256] float32 output.

Strategy: shard the batch dim (4096) into 8 contiguous row blocks, one per
NeuronCore; replicate the tiny MLP weights.  The reference's [B, D, D]
Jacobian is never materialized — the output reduces to 6 small matmuls plus
elementwise work:

    x, v  = input[:, :D], input[:, D:]
    h     = x @ W1.T + b1;  mask = (h > 0);  a = relu(h)
    s     = sigmoid(a @ W2.T + b2)
    nsig  = (s - 1) * s              # = -s(1-s)
    p     = (v^2 * (-sgn)) * nsig    # = v^2 * sgn * sig
    term1 = ((p @ W2) * mask) @ W1
    T2p   = (mask * (v @ W1.T)) @ W2.T
    dv    = (-1/(0.5 s + C/2)) * ((v * nsig) * T2p + 0.5*sgn*term1)
    out   = concat([v, dv], axis=1)

On-chip layout is feature-major (features on SBUF partitions, batch on the
free dim); x/v are PE-transposed on load and dv transposed back on store.
All matmuls run in float32r via bitcast views (fp32 storage, 1 col/cycle).
The PE instruction stream interleaves the two batch segments so the tensor
engine never idles long enough for the HAM clock gate to re-throttle.
"""

import sys

sys.path.insert(0, "/opt/trn_rl_repo")

import numpy as np

import concourse.bass as bass  # noqa: F401
import concourse.bacc as bacc
import concourse.mybir as mybir
import concourse.tile as tile
from concourse.masks import make_identity
from concourse.bass_utils import run_bass_kernel_spmd

F32 = mybir.dt.float32
F32R = mybir.dt.float32r
AF = mybir.ActivationFunctionType
ALU = mybir.AluOpType

D = 128
CONST = 0.618
SIGN = 4
N_CORES = 8
BATCH = 4096
B = BATCH // N_CORES  # rows per core


def _build(nc, B=512, n_seg=2, h_dtype="float32r"):
    NCH = B // D           # 128-row chunks for PE transposes
    SEG = B // n_seg       # batch columns per pipeline segment
    CPS = SEG // D         # chunks per segment
    assert NCH * D == B and SEG * n_seg == B and CPS * D == SEG

    inp = nc.dram_tensor("inp", [B, 2 * D], F32, kind="ExternalInput").ap()
    W1 = nc.dram_tensor("W1", [2 * D, D], F32, kind="ExternalInput").ap()
    b1 = nc.dram_tensor("b1", [2 * D], F32, kind="ExternalInput").ap()
    W2 = nc.dram_tensor("W2", [D, 2 * D], F32, kind="ExternalInput").ap()
    b2 = nc.dram_tensor("b2", [D], F32, kind="ExternalInput").ap()
    out = nc.dram_tensor("out", [B, 2 * D], F32, kind="ExternalOutput").ap()

    def r(ap):
        return ap.bitcast(F32R)

    with tile.TileContext(nc) as tc:
        with (
            tc.tile_pool(name="consts", bufs=1) as consts,
            tc.tile_pool(name="sb", bufs=1) as sb,
            tc.tile_pool(name="seg", bufs=2) as sgp,
            tc.tile_pool(name="ps", bufs=2, space="PSUM") as ps,
            tc.tile_pool(name="pss", bufs=2, space="PSUM") as pss,
            tc.tile_pool(name="pst", bufs=2, space="PSUM") as pst,
        ):
            # ---------------- DMAs first on every queue ----------------
            # DMA issue costs ~0.65us of engine time each. scalar gets the
            # first input half + biases; sync gets W1 (needed first), the
            # second input half, W2, and the v passthrough.
            it = sb.tile([D, NCH, 2 * D], F32, tag="it", name="it")
            nc.scalar.dma_start(
                it[:, 0:2, :],
                inp.rearrange("(c p) f -> p c f", p=D)[:, 0:2, :])
            b1c = consts.tile([D, 2], F32, tag="b1c", name="b1c")
            nc.scalar.dma_start(b1c[:], b1.rearrange("(h p) -> p h", p=D))
            b2c = consts.tile([D, 1], F32, tag="b2c", name="b2c")
            nc.scalar.dma_start(b2c[:], b2.rearrange("(p o) -> p o", o=1))

            w1k = consts.tile([D, 2, D], F32, tag="w1k", name="w1k")
            nc.sync.dma_start(w1k[:], W1.rearrange("(h p) j -> p h j", p=D))
            nc.sync.dma_start(
                it[:, 2:4, :],
                inp.rearrange("(c p) f -> p c f", p=D)[:, 2:4, :])
            w2n = consts.tile([D, 2 * D], F32, tag="w2n", name="w2n")
            nc.sync.dma_start(w2n[:], W2)
            # v passthrough: HBM->HBM, independent of all compute
            nc.sync.dma_start(out[:, 0:D], inp[:, D:2 * D])

            # ---------------- constants ----------------
            ident = consts.tile([D, D], F32, tag="ident", name="ident")
            make_identity(nc, ident[:])

            # sgnc = sgn; nhsgn = -0.5*sgn; c309 = CONST/2
            sgnc = consts.tile([D, 1], F32, tag="sgnc", name="sgnc")
            nc.vector.memset(sgnc[:], 1.0)
            nc.vector.memset(sgnc[:SIGN, :], -1.0)
            nhsgn = consts.tile([D, 1], F32, tag="nhsgn", name="nhsgn")
            nc.vector.memset(nhsgn[:], -0.5)
            nc.vector.memset(nhsgn[:SIGN, :], 0.5)
            c309 = consts.tile([D, 1], F32, tag="c309", name="c309")
            nc.vector.memset(c309[:], CONST / 2.0)
            warmt = consts.tile([D, 1], F32, tag="warmt", name="warmt")

            def s_warm():
                # first scalar ACT is a sigmoid: the single table set that
                # serves {sigmoid,relu,identity,copy} loads during the
                # input DMA instead of mid-chain
                nc.scalar.activation(warmt[:], c309[:], AF.Sigmoid)

            # f32r copies of the DMA-loaded weights (t1/u matmul lhsT)
            w1kr = consts.tile([D, 2, D], F32R, tag="w1kr", name="w1kr")
            w2nr = consts.tile([D, 2 * D], F32R, tag="w2nr", name="w2nr")

            def g_wcast():
                nc.gpsimd.tensor_copy(w1kr[:], w1k[:])
                nc.gpsimd.tensor_copy(w2nr[:], w2n[:])

            # ---------------- weight transposes (PE) ----------------
            w1T = consts.tile([D, 2, D], F32R, tag="w1T", name="w1T")
            w2T = consts.tile([D, 2, D], F32R, tag="w2T", name="w2T")

            def tw1():
                tp = pst.tile([D, 2, D], F32, tag="itp", name="itp")
                nc.tensor.transpose(tp[:, 0, :], w1k[:, 0, :], ident[:])
                nc.tensor.transpose(tp[:, 1, :], w1k[:, 1, :], ident[:])
                nc.vector.tensor_copy(w1T[:], tp[:])

            def tw2():
                tp = pst.tile([D, 2, D], F32, tag="itp", name="itp")
                nc.tensor.transpose(tp[:, 0, :], w2n[:, 0:D], ident[:])
                nc.tensor.transpose(tp[:, 1, :], w2n[:, D:2 * D], ident[:])
                nc.vector.tensor_copy(w2T[:], tp[:])

            # ---------------- input transposes (PE) ----------------
            xvT = sb.tile([D, 2, B], F32R, tag="xvT", name="xvT")

            def tin(g):
                itp = pst.tile([D, 2, CPS, D], F32, tag="itp", name="itp")
                for k in range(CPS):
                    c = g * CPS + k
                    nc.tensor.transpose(itp[:, 0, k, :], it[:, c, 0:D],
                                        ident[:])
                    nc.tensor.transpose(itp[:, 1, k, :], it[:, c, D:2 * D],
                                        ident[:])
                nc.vector.tensor_copy(
                    xvT[:, :, g * SEG:(g + 1) * SEG].rearrange(
                        "p s (k d) -> p s k d", k=CPS),
                    itp[:])

            # ---------------- per-segment stages ----------------
            hps = [None] * n_seg
            wps = [None] * n_seg
            zps = [None] * n_seg
            ups = [None] * n_seg
            t1ps = [None] * n_seg
            t2ps = [None] * n_seg
            a_ = [None] * n_seg
            mask = [None] * n_seg
            s_ = [None] * n_seg
            nsig = [None] * n_seg
            vsq = [None] * n_seg
            p_ = [None] * n_seg
            r_ = [None] * n_seg
            mw = [None] * n_seg
            vs = [None] * n_seg
            a2 = [None] * n_seg
            cc = [None] * n_seg
            rp = [None] * n_seg
            r02 = [None] * n_seg
            dv = [None] * n_seg

            def xT(g):
                return xvT[:, 0, g * SEG:(g + 1) * SEG]

            def vT(g):
                return xvT[:, 1, g * SEG:(g + 1) * SEG]

            def vTf(g):
                return vT(g).bitcast(F32)

            # PE stages
            def mm_h(g):
                hps[g] = ps.tile([D, 2, SEG], F32, tag="big", name="big")
                nc.tensor.matmul(hps[g][:, 0, :], w1T[:, 0, :], xT(g))
                nc.tensor.matmul(hps[g][:, 1, :], w1T[:, 1, :], xT(g))

            def mm_w(g):
                wps[g] = ps.tile([D, 2, SEG], F32, tag="wp", name="wp")
                nc.tensor.matmul(wps[g][:, 0, :], w1T[:, 0, :], vT(g))
                nc.tensor.matmul(wps[g][:, 1, :], w1T[:, 1, :], vT(g))

            def mm_z(g):
                zps[g] = pss.tile([D, SEG], F32, tag="small", name="small")
                nc.tensor.matmul(zps[g][:], w2T[:, 0, :], a_[g][:, 0, :],
                                 start=True, stop=False)
                nc.tensor.matmul(zps[g][:], w2T[:, 1, :], a_[g][:, 1, :],
                                 start=False, stop=True)

            def mm_u(g):
                ups[g] = ps.tile([D, 2, SEG], F32, tag="big", name="big")
                nc.tensor.matmul(ups[g][:, 0, :], w2nr[:, 0:D], p_[g][:])
                nc.tensor.matmul(ups[g][:, 1, :], w2nr[:, D:2 * D], p_[g][:])

            def mm_t1(g):
                t1ps[g] = pss.tile([D, SEG], F32, tag="small", name="small")
                nc.tensor.matmul(t1ps[g][:], w1kr[:, 0, :], r_[g][:, 0, :],
                                 start=True, stop=False)
                nc.tensor.matmul(t1ps[g][:], w1kr[:, 1, :], r_[g][:, 1, :],
                                 start=False, stop=True)

            def mm_t2(g):
                t2ps[g] = pss.tile([D, SEG], F32, tag="small", name="small")
                nc.tensor.matmul(t2ps[g][:], w2T[:, 0, :], mw[g][:, 0, :],
                                 start=True, stop=False)
                nc.tensor.matmul(t2ps[g][:], w2T[:, 1, :], mw[g][:, 1, :],
                                 start=False, stop=True)

            # scalar stages
            def s_relu(g):
                a_[g] = sgp.tile([D, 2, SEG], F32R, tag="a", name="a")
                for h in range(2):
                    nc.scalar.activation(a_[g][:, h, :], hps[g][:, h, :],
                                         AF.Relu, bias=b1c[:, h:h + 1])

            def s_sig(g):
                s_[g] = sgp.tile([D, SEG], F32, tag="s", name="s")
                nc.scalar.activation(s_[g][:], zps[g][:], AF.Sigmoid,
                                     bias=b2c[:, 0:1])

            def s_rp(g):
                # rp = 0.5 s + CONST/2; recip gives 1/g_abs
                rp[g] = sgp.tile([D, SEG], F32, tag="rp", name="rp")
                nc.scalar.activation(rp[g][:], s_[g][:], AF.Identity,
                                     bias=c309[:, 0:1], scale=0.5)

            # gpsimd stages
            def v_mask(g):
                # a = relu(h) >= 0, so (a > 0) == (h > 0)
                mask[g] = sgp.tile([D, 2, SEG], F32, tag="mask", name="mask")
                nc.vector.tensor_single_scalar(
                    mask[g].rearrange("p s b -> p (s b)"),
                    a_[g].rearrange("p s b -> p (s b)").bitcast(F32), 0.0,
                    ALU.is_gt)

            def v_mw(g):
                mw[g] = sgp.tile([D, 2, SEG], F32R, tag="mw", name="mw")
                nc.vector.tensor_mul(mw[g].rearrange("p s b -> p (s b)"),
                                     wps[g].rearrange("p s b -> p (s b)"),
                                     mask[g].rearrange("p s b -> p (s b)"))

            # vector chain stages
            def v_nsig(g):
                # nsig = (s-1)*s = -s(1-s)
                nsig[g] = sgp.tile([D, SEG], F32, tag="nsig", name="nsig")
                nc.vector.scalar_tensor_tensor(nsig[g][:], s_[g][:], 1.0,
                                               s_[g][:], ALU.subtract,
                                               ALU.mult)

            def v_vs(g):
                # vs = (v * -1) * nsig = v*sig
                vs[g] = sgp.tile([D, SEG], F32, tag="vs", name="vs")
                nc.vector.scalar_tensor_tensor(vs[g][:], vTf(g), -1.0,
                                               nsig[g][:], ALU.mult, ALU.mult)

            def v_p(g):
                # p = (vs * sgn) * v = v^2 sgn sig
                p_[g] = sgp.tile([D, SEG], F32R, tag="p", name="p")
                nc.vector.scalar_tensor_tensor(p_[g][:], vs[g][:],
                                               sgnc[:, 0:1], vTf(g),
                                               ALU.mult, ALU.mult)

            def v_recip(g):
                r02[g] = sgp.tile([D, SEG], F32, tag="r02", name="r02")
                nc.vector.reciprocal_approx_fast(r02[g][:], rp[g][:])

            def v_dv(g):
                dv[g] = sgp.tile([D, SEG], F32, tag="dv", name="dv")
                nc.vector.tensor_mul(dv[g][:], cc[g][:], r02[g][:])

            def v_r(g):
                r_[g] = sgp.tile([D, 2, SEG], F32R, tag="r", name="r")
                nc.vector.tensor_mul(r_[g].rearrange("p s b -> p (s b)"),
                                     ups[g].rearrange("p s b -> p (s b)"),
                                     mask[g].rearrange("p s b -> p (s b)"))

            def v_a2(g):
                a2[g] = sgp.tile([D, SEG], F32, tag="a2", name="a2")
                nc.vector.tensor_mul(a2[g][:], vs[g][:], t2ps[g][:])

            def v_cc(g):
                # cc = (t1 * (-0.5 sgn)) + v sig T2p
                cc[g] = sgp.tile([D, SEG], F32, tag="cc", name="cc")
                nc.vector.scalar_tensor_tensor(cc[g][:], t1ps[g][:],
                                               nhsgn[:, 0:1], a2[g][:],
                                               ALU.mult, ALU.add)

            # output transposes + stores
            ot = sb.tile([D, NCH, D], F32, tag="ot", name="ot")

            def tout(g):
                otp = pst.tile([D, 2, D], F32, tag="itp", name="itp")
                for k in range(CPS):
                    nc.tensor.transpose(otp[:, k, :],
                                        dv[g][:, k * D:(k + 1) * D], ident[:])
                nc.scalar.copy(ot[:, g * CPS:(g + 1) * CPS, :], otp[:])
                nc.sync.dma_start(
                    out.rearrange("(c p) f -> p c f", p=D)
                    [:, g * CPS:(g + 1) * CPS, D:2 * D],
                    ot[:, g * CPS:(g + 1) * CPS, :])

            # ---------------- schedule (program order per engine) -------
            s_warm()
            tw1()
            g_wcast()
            tin(0)
            mm_h(0)
            tw2()
            tin(1)
            s_relu(0)
            mm_h(1)
            v_mask(0)
            mm_z(0)
            s_relu(1)
            mm_w(0)
            s_sig(0)
            v_nsig(0)
            v_vs(0)
            v_p(0)
            mm_z(1)
            s_sig(1)
            mm_w(1)
            mm_u(0)
            s_rp(0)
            v_mw(0)
            v_r(0)
            v_nsig(1)
            v_vs(1)
            v_p(1)
            mm_u(1)
            mm_t1(0)
            mm_t2(0)
            s_rp(1)
            v_mask(1)
            v_mw(1)
            v_r(1)
            mm_t1(1)
            mm_t2(1)
            v_recip(0)
            v_a2(0)
            v_cc(0)
            v_dv(0)
            tout(0)
            v_recip(1)
            v_a2(1)
            v_cc(1)
            v_dv(1)
            tout(1)

    return nc


_CACHE = {}


def _get_nc(variant="v2"):
    if variant not in _CACHE:
        nc = bacc.Bacc("TRN2", target_bir_lowering=False, debug=False,
                       num_devices=N_CORES)
        _build(nc, B=B)
        nc.compile()
        _CACHE[variant] = nc
    return _CACHE[variant]


def kernel(t, input_, W1, b1, W2, b2):
    input_ = np.ascontiguousarray(np.asarray(input_, dtype=np.float32))
    W1 = np.ascontiguousarray(np.asarray(W1, dtype=np.float32))
    b1 = np.ascontiguousarray(np.asarray(b1, dtype=np.float32))
    W2 = np.ascontiguousarray(np.asarray(W2, dtype=np.float32))
    b2 = np.ascontiguousarray(np.asarray(b2, dtype=np.float32))
    assert input_.shape == (BATCH, 2 * D)

    nc = _get_nc()
    in_maps = [
        {"inp": input_[c * B:(c + 1) * B], "W1": W1, "b1": b1, "W2": W2, "b2": b2}
        for c in range(N_CORES)
    ]
    res = run_bass_kernel_spmd(nc, in_maps, core_ids=list(range(N_CORES)))
    return np.concatenate([res.results[c]["out"] for c in range(N_CORES)], axis=0)
